# revision 40
# baseline (speedup 1.0000x reference)
"""Causal self-attention on 8 TRN2 NeuronCores.

Sharding: core c -> (batch b = c//2, head-group g = c%2).
B=4, T=2048, D=1024, 16 heads x 64. Each core computes attention for its
batch and its 8 heads, plus the partial output projection for those heads;
the host sums the two partial projections per batch.

Device layouts (host pre-transposes and converts to bf16):
  xT    [1024, 2048] bf16  x[b].T
  wqkT  [1024, 1024] bf16  cols 0..511 q-feats, 512..1023 k-feats (group g)
  wvT   [1024, 512]  bf16  v-feats (group g)
  wpT   [512, 1024]  bf16  w_proj[:, g*512:(g+1)*512].T
  tri   [128, 256]   bf16  tri[k, q] = 1 if (q % 128) >= k else 0 (doubled)
  ones8 [128, 8]     bf16  ones for the V denominator columns
Output: yT [1024, 2048] bf16 partial y[b].T.

Heads are processed as 4 pairs; the even head of a pair lives on SBUF
partitions 0-63 and the odd head on 64-127, so the K=64 score matmuls of the
two heads land on disjoint PE row groups (tile_position (0,0)/(64,0)) and are
emitted adjacently to execute concurrently in the systolic array. Diagonal
512x512 blocks are computed/exp'd only on causal column ranges (widths
512/384/256/128 packed as [512|384] and [256|128] PSUM tiles), leaving one
[128,2,128] strided triangular mask multiply per diagonal group (tri holds
the pattern twice so two blocks mask in one DVE op). exp(x/8) runs on the
ACT engine straight out of PSUM. Softmax denominators ride as a 65th row of
the PV matmul (ones column in V); normalization is deferred off the PSUM
critical path: a high-priority ACT copy of the 64 value rows plus a DVE copy
of the denominator row release the av bank quickly, then DVE fast-reciprocal,
GpSimd partition-broadcast, and a DVE multiply write normalized bf16 out_t.
Phase-1 projections for head pairs 1-3 are emitted between attends so their
matmuls fill the PE while attention waits on exp; phase 3 is n-outer for the
same reason. Output is bf16 partials summed on the host in fp32.
"""

import sys

for _p in ("/opt/pypackages", "/opt/trn_rl_repo"):
    if _p not in sys.path:
        sys.path.insert(0, _p)

from contextlib import ExitStack

import ml_dtypes
import numpy as np

import concourse.tile as tile
from concourse import bacc, mybir
from concourse.bass_utils import run_bass_kernel_spmd

F32 = mybir.dt.float32
BF16 = mybir.dt.bfloat16
AF = mybir.ActivationFunctionType
OP = mybir.AluOpType

D = 1024
T = 2048
NH_LOC = 8          # heads per core
DH = 64
GF = NH_LOC * DH    # 512 features per group

LAST_RESULTS = None
_CACHED = None


def build_program():
    nc = bacc.Bacc("TRN2", target_bir_lowering=False, debug=False)

    xT_d = nc.dram_tensor("xT", [D, T], BF16, kind="ExternalInput").ap()
    wqk_d = nc.dram_tensor("wqkT", [D, 2 * GF], BF16, kind="ExternalInput").ap()
    wv_d = nc.dram_tensor("wvT", [D, GF], BF16, kind="ExternalInput").ap()
    wp_d = nc.dram_tensor("wpT", [GF, D], BF16, kind="ExternalInput").ap()
    tri_d = nc.dram_tensor("tri", [128, 256], BF16, kind="ExternalInput").ap()
    ones_d = nc.dram_tensor("ones8", [128, 8], BF16, kind="ExternalInput").ap()
    yT_d = nc.dram_tensor("yT", [D, T], BF16, kind="ExternalOutput").ap()

    with tile.TileContext(nc) as tc:
        with ExitStack() as octx:
            # ---- persistent SBUF pools ---------------------------------
            w_pool = octx.enter_context(tc.tile_pool(name="weights", bufs=1))
            x_pool = octx.enter_context(tc.tile_pool(name="xT", bufs=1))
            qk_pool = octx.enter_context(tc.tile_pool(name="qkT", bufs=1))
            v_pool = octx.enter_context(tc.tile_pool(name="vN", bufs=1))
            o_pool = octx.enter_context(tc.tile_pool(name="outT", bufs=1))
            pt_pool = octx.enter_context(tc.tile_pool(name="pt", bufs=6))
            r_pool = octx.enter_context(tc.tile_pool(name="recip", bufs=4))
            y_pool = octx.enter_context(tc.tile_pool(name="y", bufs=4))
            # ---- PSUM pools: 3*2 + 2 = 8 banks -------------------------
            ps = octx.enter_context(tc.tile_pool(name="ps", bufs=3, space="PSUM"))
            ps_av = octx.enter_context(tc.tile_pool(name="ps_av", bufs=2, space="PSUM"))

            wqk_t = [w_pool.tile([128, 2 * GF], BF16, name=f"wqk{k}", tag=f"wqk{k}") for k in range(8)]
            wv_all = w_pool.tile([128, 8, GF], BF16, name="wva", tag="wva")
            wp_all = w_pool.tile([128, 4, D], BF16, name="wpa", tag="wpa")
            wv_t = [wv_all[:, k, :] for k in range(8)]
            wp_t = [wp_all[:, k, :] for k in range(4)]
            tri_t = w_pool.tile([128, 256], BF16, name="tri", tag="tri")
            x_t = [x_pool.tile([128, T], BF16, name=f"x{k}", tag=f"x{k}") for k in range(8)]
            qk_t = [qk_pool.tile([128, T], BF16, name=f"qk{m}", tag=f"qk{m}") for m in range(8)]
            v_t = [v_pool.tile([128, 8 * (DH + 1)], BF16, name=f"v{t}", tag=f"v{t}") for t in range(16)]
            out_t = [o_pool.tile([128, T], BF16, name=f"o{k}", tag=f"o{k}") for k in range(4)]

            # ---- input DMAs: batched weight transfers (one dispatch
            #      each), x tiles split across both HWDGE queues ----------
            for k in range(8):
                eng = nc.sync if k % 2 == 0 else nc.scalar
                eng.dma_start(wqk_t[k][:], wqk_d[k * 128:(k + 1) * 128, :])
                eng.dma_start(x_t[k][:], xT_d[k * 128:(k + 1) * 128, :])
            for t in range(16):
                dst = v_t[t][:].rearrange("p (h e) -> p h e", h=8, e=65)[:, :, 64:65]
                nc.scalar.dma_start(dst, ones_d[:].unsqueeze(2))
            nc.scalar.dma_start(tri_t[:], tri_d[:])
            nc.sync.dma_start(
                wv_all[:], wv_d[:].rearrange("(k p) f -> p k f", k=8, p=128))
            nc.sync.dma_start(
                wp_all[:], wp_d[:].rearrange("(k p) f -> p k f", k=4, p=128))

            # ================= phase 1: qkv projections =================
            def qk_feats_n(m, n):
                # q/k features of pair-tile m (m 0-3: q, m 4-7: k)
                off = m * 128 if m < 4 else 512 + (m - 4) * 128
                pg = ps.tile([128, 512], F32, name="psg", tag="ps")
                for k in range(8):
                    nc.tensor.matmul(
                        pg[:], wqk_t[k][:, off:off + 128],
                        x_t[k][:, n * 512:(n + 1) * 512],
                        start=(k == 0), stop=(k == 7),
                        skip_group_check=True,
                    )
                dst = qk_t[m][:, n * 512:(n + 1) * 512]
                nc.vector.tensor_copy(dst, pg[:])

            def qk_feats(m):
                for n in range(4):
                    qk_feats_n(m, n)

            def v_feats(tt):
                pg = ps.tile([128, 512], F32, name="psg", tag="ps")
                for k in range(8):
                    nc.tensor.matmul(
                        pg[:], x_t[k][:, tt * 128:(tt + 1) * 128], wv_t[k][:],
                        start=(k == 0), stop=(k == 7),
                        skip_group_check=True,
                    )
                src = pg[:].rearrange("p (h e) -> p h e", h=8, e=64)
                dst = v_t[tt][:].rearrange("p (h e) -> p h e", h=8, e=65)[:, :, 0:64]
                nc.vector.tensor_copy(dst, src)

            def phase3_n(n):
                # output projection for column chunk n (needs out_t[:, n*512]
                # of every head pair, i.e. attend c=n of all four pairs)
                for m in range(8):
                    psy = ps.tile([128, 512], F32, name="psg", tag="ps")
                    for kk in range(4):
                        nc.tensor.matmul(
                            psy[:], wp_t[kk][:, m * 128:(m + 1) * 128],
                            out_t[kk][:, n * 512:(n + 1) * 512],
                            start=(kk == 0), stop=(kk == 3),
                            skip_group_check=True,
                        )
                    yt = y_pool.tile([128, 512], BF16, name="yst", tag="yst")
                    if m % 2 == 0:
                        nc.vector.tensor_copy(yt[:], psy[:])
                    else:
                        nc.scalar.activation(yt[:], psy[:], AF.Copy)
                    nc.sync.dma_start(
                        yT_d[m * 128:(m + 1) * 128, n * 512:(n + 1) * 512], yt[:]
                    )

            # pair 0 q/k first (n outer so the first accumulation chains
            # start as soon as the early x tiles land), then all v, then
            # remaining pairs (emitted interleaved with attention below).
            for n in range(4):
                for m in (0, 4):
                    qk_feats_n(m, n)
            for tt in range(16):
                v_feats(tt)

            # ================= phase 2: causal attention ================
            def attend_c(hp, c):
                qT = qk_t[hp]
                kT = qk_t[4 + hp]
                vsl = [slice((2 * hp + par) * 65, (2 * hp + par) * 65 + 65)
                       for par in range(2)]
                if True:
                    avs = [ps_av.tile([65, 512], F32, name="av", tag="av")
                           for _ in range(2)]

                    def s_mm(par, st_, j, qlo, lo, hi):
                        r0 = par * 64
                        nc.tensor.matmul(
                            st_[:, lo:hi],
                            kT[r0:r0 + 64, j * 128:(j + 1) * 128],
                            qT[r0:r0 + 64, c * 512 + qlo:(c + 1) * 512],
                            start=True, stop=True, skip_group_check=True,
                        )

                    def pv_mm(par, j, pt_ap, colo, last):
                        nc.tensor.matmul(
                            avs[par][:, colo:512], v_t[j][:, vsl[par]], pt_ap,
                            start=(j == 0), stop=last, skip_group_check=True,
                        )

                    # --- off-diagonal key blocks, two at a time ---------
                    for w in range(0, 4 * c, 2):
                        ss = [ps.tile([128, 1024], F32, name="s", tag="ps")
                              for _ in range(2)]
                        for par in range(2):
                            for idx in range(2):
                                s_mm(par, ss[par], w + idx, 0, idx * 512, idx * 512 + 512)
                        pts = [pt_pool.tile([128, 1024], BF16, name="pt", tag="pt")
                               for _ in range(2)]
                        nc.scalar.activation(pts[0][:], ss[0][:], AF.Exp, scale=0.125)
                        nc.scalar.activation(pts[1][:], ss[1][:], AF.Exp, scale=0.125)
                        for par in range(2):
                            for idx in range(2):
                                pv_mm(par, w + idx, pts[par][:, idx * 512:idx * 512 + 512], 0, False)

                    # --- diagonal quad: causal widths packed [512|384],
                    #     [256|128]; one [128,2,128] tri-mask per group ---
                    for p0, sl0, sl1 in ((0, (0, 512), (512, 896)),
                                         (2, (0, 256), (256, 384))):
                        ss = [ps.tile([128, 1024], F32, name="s", tag="ps")
                              for _ in range(2)]
                        pts = [pt_pool.tile([128, 1024], BF16, name="pt", tag="pt")
                               for _ in range(2)]
                        for par in range(2):
                            for pp, (lo, hi) in ((p0, sl0), (p0 + 1, sl1)):
                                s_mm(par, ss[par], 4 * c + pp, 128 * pp, lo, hi)
                        for par in range(2):
                            nc.scalar.activation(
                                pts[par][:, sl0[0]:sl1[1]], ss[par][:, sl0[0]:sl1[1]],
                                AF.Exp, scale=0.125,
                            )
                        wgrp = sl0[1] - sl0[0]          # 512 (d1) or 256 (d2)
                        for par in range(2):
                            slab = pts[par][:, 0:2 * wgrp].rearrange(
                                "p (b w) -> p b w", b=2, w=wgrp)[:, :, 0:128]
                            trib = tri_t[:].rearrange(
                                "p (b w) -> p b w", b=2, w=128)
                            nc.vector.tensor_tensor(slab, slab, trib, op=OP.mult)
                        for par in range(2):
                            for pp, (lo, hi) in ((p0, sl0), (p0 + 1, sl1)):
                                pv_mm(par, 4 * c + pp, pts[par][:, lo:hi],
                                      128 * pp, pp == 3)

                    # --- normalization: av released after two short
                    #     copies; recip/mult run off the critical path -----
                    for par in range(2):
                        av = avs[par]
                        r0 = par * 64
                        with tc.high_priority():
                            st_ = r_pool.tile([64, 512], F32, name="st", tag="st")
                            nc.scalar.activation(st_[:], av[0:64, :], AF.Copy)
                            den = r_pool.tile([1, 512], F32, name="den", tag="den")
                            nc.vector.tensor_copy(den[:], av[64:65, :])
                        rec = r_pool.tile([1, 512], F32, name="rec", tag="rec")
                        nc.vector.reciprocal_approx_fast(rec[:], den[:])
                        rb = r_pool.tile([64, 512], F32, name="rb", tag="rb")
                        nc.gpsimd.partition_broadcast(rb[:], rec[:])
                        nc.vector.tensor_tensor(
                            out_t[hp][r0:r0 + 64, c * 512:(c + 1) * 512],
                            st_[:], rb[:], op=OP.mult,
                        )

            def attend(hp):
                for c in range(4):
                    attend_c(hp, c)

            attend(0)
            qk_feats(1)
            qk_feats(5)
            attend(1)
            qk_feats(2)
            qk_feats(6)
            attend(2)
            qk_feats(3)
            qk_feats(7)
            attend(3)

            # ================= phase 3: output projection ===============
            # n-outer: chunk n becomes ready as attend(3) finishes c=n
            for n in range(4):
                phase3_n(n)

    nc.compile()
    return nc


def kernel(x, w_qkv, w_proj):
    global LAST_RESULTS, _CACHED
    x = np.asarray(x, dtype=np.float32)
    w_qkv = np.asarray(w_qkv, dtype=np.float32)
    w_proj = np.asarray(w_proj, dtype=np.float32)
    B = x.shape[0]

    if _CACHED is None:
        _CACHED = build_program()
    nc = _CACHED

    kk, qq = np.arange(128)[:, None], np.arange(128)[None, :]
    tri1 = (qq >= kk).astype(ml_dtypes.bfloat16)
    tri = np.concatenate([tri1, tri1], axis=1)
    in_maps = []
    for c in range(8):
        b, g = c // 2, c % 2
        wq = w_qkv[g * GF:(g + 1) * GF, :]                # [512, 1024]
        wk = w_qkv[D + g * GF: D + (g + 1) * GF, :]
        wv = w_qkv[2 * D + g * GF: 2 * D + (g + 1) * GF, :]
        in_maps.append({
            "xT": np.ascontiguousarray(x[b].T).astype(ml_dtypes.bfloat16),
            "wqkT": np.ascontiguousarray(np.concatenate([wq, wk], axis=0).T).astype(ml_dtypes.bfloat16),
            "wvT": np.ascontiguousarray(wv.T).astype(ml_dtypes.bfloat16),
            "wpT": np.ascontiguousarray(w_proj[:, g * GF:(g + 1) * GF].T).astype(ml_dtypes.bfloat16),
            "tri": tri,
            "ones8": np.ones((128, 8), ml_dtypes.bfloat16),
        })

    res = run_bass_kernel_spmd(nc, in_maps, core_ids=list(range(8)))
    LAST_RESULTS = res

    y = np.empty_like(x)
    for b in range(B):
        yT = (res.results[2 * b]["yT"].astype(np.float32)
              + res.results[2 * b + 1]["yT"].astype(np.float32))
        y[b] = yT.T
    return y


# revision 41
# speedup vs baseline: 1.0030x; 1.0030x over previous
"""Causal self-attention on 8 TRN2 NeuronCores.

Sharding: core c -> (batch b = c//2, head-group g = c%2).
B=4, T=2048, D=1024, 16 heads x 64. Each core computes attention for its
batch and its 8 heads, plus the partial output projection for those heads;
the host sums the two partial projections per batch.

Device layouts (host pre-transposes and converts to bf16):
  xT    [1024, 2048] bf16  x[b].T
  wqkT  [1024, 1024] bf16  cols 0..511 q-feats, 512..1023 k-feats (group g)
  wvT   [1024, 512]  bf16  v-feats (group g)
  wpT   [512, 1024]  bf16  w_proj[:, g*512:(g+1)*512].T
  tri   [128, 256]   bf16  tri[k, q] = 1 if (q % 128) >= k else 0 (doubled)
  ones8 [128, 8]     bf16  ones for the V denominator columns
Output: yT [1024, 2048] bf16 partial y[b].T.

Heads are processed as 4 pairs; the even head of a pair lives on SBUF
partitions 0-63 and the odd head on 64-127, so the K=64 score matmuls of the
two heads land on disjoint PE row groups (tile_position (0,0)/(64,0)) and are
emitted adjacently to execute concurrently in the systolic array. Diagonal
512x512 blocks are computed/exp'd only on causal column ranges (widths
512/384/256/128 packed as [512|384] and [256|128] PSUM tiles), leaving one
[128,2,128] strided triangular mask multiply per diagonal group (tri holds
the pattern twice so two blocks mask in one DVE op). exp(x/8) runs on the
ACT engine straight out of PSUM. Softmax denominators ride as a 65th row of
the PV matmul (ones column in V); normalization is deferred off the PSUM
critical path: a high-priority ACT copy of the 64 value rows plus a DVE copy
of the denominator row release the av bank quickly, then DVE fast-reciprocal,
GpSimd partition-broadcast, and a DVE multiply write normalized bf16 out_t.
Phase-1 projections for head pairs 1-3 are emitted between attends so their
matmuls fill the PE while attention waits on exp; phase 3 is n-outer for the
same reason. Output is bf16 partials summed on the host in fp32.
"""

import sys

for _p in ("/opt/pypackages", "/opt/trn_rl_repo"):
    if _p not in sys.path:
        sys.path.insert(0, _p)

from contextlib import ExitStack

import ml_dtypes
import numpy as np

import concourse.tile as tile
from concourse import bacc, mybir
from concourse.bass_utils import run_bass_kernel_spmd

F32 = mybir.dt.float32
BF16 = mybir.dt.bfloat16
AF = mybir.ActivationFunctionType
OP = mybir.AluOpType

D = 1024
T = 2048
NH_LOC = 8          # heads per core
DH = 64
GF = NH_LOC * DH    # 512 features per group

LAST_RESULTS = None
_CACHED = None


def build_program():
    nc = bacc.Bacc("TRN2", target_bir_lowering=False, debug=False)

    xT_d = nc.dram_tensor("xT", [D, T], BF16, kind="ExternalInput").ap()
    wqk_d = nc.dram_tensor("wqkT", [D, 2 * GF], BF16, kind="ExternalInput").ap()
    wv_d = nc.dram_tensor("wvT", [D, GF], BF16, kind="ExternalInput").ap()
    wp_d = nc.dram_tensor("wpT", [GF, D], BF16, kind="ExternalInput").ap()
    tri_d = nc.dram_tensor("tri", [128, 256], BF16, kind="ExternalInput").ap()
    ones_d = nc.dram_tensor("ones8", [128, 8], BF16, kind="ExternalInput").ap()
    yT_d = nc.dram_tensor("yT", [D, T], BF16, kind="ExternalOutput").ap()

    with tile.TileContext(nc) as tc:
        with ExitStack() as octx:
            # ---- persistent SBUF pools ---------------------------------
            w_pool = octx.enter_context(tc.tile_pool(name="weights", bufs=1))
            x_pool = octx.enter_context(tc.tile_pool(name="xT", bufs=1))
            qk_pool = octx.enter_context(tc.tile_pool(name="qkT", bufs=1))
            v_pool = octx.enter_context(tc.tile_pool(name="vN", bufs=1))
            o_pool = octx.enter_context(tc.tile_pool(name="outT", bufs=1))
            pt_pool = octx.enter_context(tc.tile_pool(name="pt", bufs=8))
            r_pool = octx.enter_context(tc.tile_pool(name="recip", bufs=4))
            y_pool = octx.enter_context(tc.tile_pool(name="y", bufs=4))
            # ---- PSUM pools: 3*2 + 2 = 8 banks -------------------------
            ps = octx.enter_context(tc.tile_pool(name="ps", bufs=3, space="PSUM"))
            ps_av = octx.enter_context(tc.tile_pool(name="ps_av", bufs=2, space="PSUM"))

            wqk_t = [w_pool.tile([128, 2 * GF], BF16, name=f"wqk{k}", tag=f"wqk{k}") for k in range(8)]
            wv_all = w_pool.tile([128, 8, GF], BF16, name="wva", tag="wva")
            wp_all = w_pool.tile([128, 4, D], BF16, name="wpa", tag="wpa")
            wv_t = [wv_all[:, k, :] for k in range(8)]
            wp_t = [wp_all[:, k, :] for k in range(4)]
            tri_t = w_pool.tile([128, 256], BF16, name="tri", tag="tri")
            x_t = [x_pool.tile([128, T], BF16, name=f"x{k}", tag=f"x{k}") for k in range(8)]
            qk_t = [qk_pool.tile([128, T], BF16, name=f"qk{m}", tag=f"qk{m}") for m in range(8)]
            v_t = [v_pool.tile([128, 8 * (DH + 1)], BF16, name=f"v{t}", tag=f"v{t}") for t in range(16)]
            out_t = [o_pool.tile([128, T], BF16, name=f"o{k}", tag=f"o{k}") for k in range(4)]

            # ---- input DMAs: batched weight transfers (one dispatch
            #      each), x tiles split across both HWDGE queues ----------
            for k in range(8):
                eng = nc.sync if k % 2 == 0 else nc.scalar
                eng.dma_start(wqk_t[k][:], wqk_d[k * 128:(k + 1) * 128, :])
                eng.dma_start(x_t[k][:], xT_d[k * 128:(k + 1) * 128, :])
            for t in range(16):
                dst = v_t[t][:].rearrange("p (h e) -> p h e", h=8, e=65)[:, :, 64:65]
                nc.scalar.dma_start(dst, ones_d[:].unsqueeze(2))
            nc.scalar.dma_start(tri_t[:], tri_d[:])
            nc.sync.dma_start(
                wv_all[:], wv_d[:].rearrange("(k p) f -> p k f", k=8, p=128))
            nc.sync.dma_start(
                wp_all[:], wp_d[:].rearrange("(k p) f -> p k f", k=4, p=128))

            # ================= phase 1: qkv projections =================
            def qk_feats_n(m, n):
                # q/k features of pair-tile m (m 0-3: q, m 4-7: k)
                off = m * 128 if m < 4 else 512 + (m - 4) * 128
                pg = ps.tile([128, 512], F32, name="psg", tag="ps")
                for k in range(8):
                    nc.tensor.matmul(
                        pg[:], wqk_t[k][:, off:off + 128],
                        x_t[k][:, n * 512:(n + 1) * 512],
                        start=(k == 0), stop=(k == 7),
                        skip_group_check=True,
                    )
                dst = qk_t[m][:, n * 512:(n + 1) * 512]
                nc.vector.tensor_copy(dst, pg[:])

            def qk_feats(m):
                for n in range(4):
                    qk_feats_n(m, n)

            def v_feats(tt):
                pg = ps.tile([128, 512], F32, name="psg", tag="ps")
                for k in range(8):
                    nc.tensor.matmul(
                        pg[:], x_t[k][:, tt * 128:(tt + 1) * 128], wv_t[k][:],
                        start=(k == 0), stop=(k == 7),
                        skip_group_check=True,
                    )
                src = pg[:].rearrange("p (h e) -> p h e", h=8, e=64)
                dst = v_t[tt][:].rearrange("p (h e) -> p h e", h=8, e=65)[:, :, 0:64]
                nc.vector.tensor_copy(dst, src)

            def phase3_n(n):
                # output projection for column chunk n (needs out_t[:, n*512]
                # of every head pair, i.e. attend c=n of all four pairs)
                for m in range(8):
                    psy = ps.tile([128, 512], F32, name="psg", tag="ps")
                    for kk in range(4):
                        nc.tensor.matmul(
                            psy[:], wp_t[kk][:, m * 128:(m + 1) * 128],
                            out_t[kk][:, n * 512:(n + 1) * 512],
                            start=(kk == 0), stop=(kk == 3),
                            skip_group_check=True,
                        )
                    yt = y_pool.tile([128, 512], BF16, name="yst", tag="yst")
                    if m % 2 == 0:
                        nc.vector.tensor_copy(yt[:], psy[:])
                    else:
                        nc.scalar.activation(yt[:], psy[:], AF.Copy)
                    nc.sync.dma_start(
                        yT_d[m * 128:(m + 1) * 128, n * 512:(n + 1) * 512], yt[:]
                    )

            # pair 0 q/k first (n outer so the first accumulation chains
            # start as soon as the early x tiles land), then all v, then
            # remaining pairs (emitted interleaved with attention below).
            for n in range(4):
                for m in (0, 4):
                    qk_feats_n(m, n)
            for tt in range(16):
                v_feats(tt)

            # ================= phase 2: causal attention ================
            def attend_c(hp, c):
                qT = qk_t[hp]
                kT = qk_t[4 + hp]
                vsl = [slice((2 * hp + par) * 65, (2 * hp + par) * 65 + 65)
                       for par in range(2)]
                avs = [ps_av.tile([65, 512], F32, name="av", tag="av")
                       for _ in range(2)]

                def s_mm(par, st_, j, qlo, lo, hi):
                    r0 = par * 64
                    nc.tensor.matmul(
                        st_[:, lo:hi],
                        kT[r0:r0 + 64, j * 128:(j + 1) * 128],
                        qT[r0:r0 + 64, c * 512 + qlo:(c + 1) * 512],
                        start=True, stop=True, skip_group_check=True,
                    )

                def emit_pvs(unit, pts):
                    for par in range(2):
                        for j, qlo, lo, hi in unit['js']:
                            nc.tensor.matmul(
                                avs[par][:, qlo:512], v_t[j][:, vsl[par]],
                                pts[par][:, lo:hi],
                                start=(j == 0), stop=(j == 4 * c + 3),
                                skip_group_check=True,
                            )

                units = [{'js': [(w, 0, 0, 512), (w + 1, 0, 512, 1024)],
                          'diag': False} for w in range(0, 4 * c, 2)]
                units.append({'js': [(4 * c, 0, 0, 512),
                                     (4 * c + 1, 128, 512, 896)],
                              'diag': True, 'w': 512})
                units.append({'js': [(4 * c + 2, 256, 0, 256),
                                     (4 * c + 3, 384, 256, 384)],
                              'diag': True, 'w': 256})

                # PV matmuls lag their unit by one so their exps are long
                # finished when they reach the strict-FIFO PE queue.
                prev = None
                for unit in units:
                    if prev is not None:
                        emit_pvs(*prev)
                    ss = [ps.tile([128, 1024], F32, name="s", tag="ps")
                          for _ in range(2)]
                    for par in range(2):
                        for j, qlo, lo, hi in unit['js']:
                            s_mm(par, ss[par], j, qlo, lo, hi)
                    pts = [pt_pool.tile([128, 1024], BF16, name="pt", tag="pt")
                           for _ in range(2)]
                    lo0, hi1 = unit['js'][0][2], unit['js'][1][3]
                    for par in range(2):
                        nc.scalar.activation(pts[par][:, lo0:hi1],
                                             ss[par][:, lo0:hi1],
                                             AF.Exp, scale=0.125)
                    if unit['diag']:
                        wg = unit['w']
                        for par in range(2):
                            slab = pts[par][:, 0:2 * wg].rearrange(
                                "p (b w) -> p b w", b=2, w=wg)[:, :, 0:128]
                            trib = tri_t[:].rearrange(
                                "p (b w) -> p b w", b=2, w=128)
                            nc.vector.tensor_tensor(slab, slab, trib, op=OP.mult)
                    prev = (unit, pts)
                emit_pvs(*prev)

                # --- normalization: av released after two short copies;
                #     recip/mult run off the critical path ----------------
                for par in range(2):
                    av = avs[par]
                    r0 = par * 64
                    with tc.high_priority():
                        st_ = r_pool.tile([64, 512], F32, name="st", tag="st")
                        nc.scalar.activation(st_[:], av[0:64, :], AF.Copy)
                        den = r_pool.tile([1, 512], F32, name="den", tag="den")
                        nc.vector.tensor_copy(den[:], av[64:65, :])
                    rec = r_pool.tile([1, 512], F32, name="rec", tag="rec")
                    nc.vector.reciprocal_approx_fast(rec[:], den[:])
                    rb = r_pool.tile([64, 512], F32, name="rb", tag="rb")
                    nc.gpsimd.partition_broadcast(rb[:], rec[:])
                    nc.vector.tensor_tensor(
                        out_t[hp][r0:r0 + 64, c * 512:(c + 1) * 512],
                        st_[:], rb[:], op=OP.mult,
                    )

            def attend(hp):
                for c in range(4):
                    attend_c(hp, c)

            attend(0)
            qk_feats(1)
            qk_feats(5)
            attend(1)
            qk_feats(2)
            qk_feats(6)
            attend(2)
            qk_feats(3)
            qk_feats(7)
            attend(3)

            # ================= phase 3: output projection ===============
            # n-outer: chunk n becomes ready as attend(3) finishes c=n
            for n in range(4):
                phase3_n(n)

    nc.compile()
    return nc


def kernel(x, w_qkv, w_proj):
    global LAST_RESULTS, _CACHED
    x = np.asarray(x, dtype=np.float32)
    w_qkv = np.asarray(w_qkv, dtype=np.float32)
    w_proj = np.asarray(w_proj, dtype=np.float32)
    B = x.shape[0]

    if _CACHED is None:
        _CACHED = build_program()
    nc = _CACHED

    kk, qq = np.arange(128)[:, None], np.arange(128)[None, :]
    tri1 = (qq >= kk).astype(ml_dtypes.bfloat16)
    tri = np.concatenate([tri1, tri1], axis=1)
    in_maps = []
    for c in range(8):
        b, g = c // 2, c % 2
        wq = w_qkv[g * GF:(g + 1) * GF, :]                # [512, 1024]
        wk = w_qkv[D + g * GF: D + (g + 1) * GF, :]
        wv = w_qkv[2 * D + g * GF: 2 * D + (g + 1) * GF, :]
        in_maps.append({
            "xT": np.ascontiguousarray(x[b].T).astype(ml_dtypes.bfloat16),
            "wqkT": np.ascontiguousarray(np.concatenate([wq, wk], axis=0).T).astype(ml_dtypes.bfloat16),
            "wvT": np.ascontiguousarray(wv.T).astype(ml_dtypes.bfloat16),
            "wpT": np.ascontiguousarray(w_proj[:, g * GF:(g + 1) * GF].T).astype(ml_dtypes.bfloat16),
            "tri": tri,
            "ones8": np.ones((128, 8), ml_dtypes.bfloat16),
        })

    res = run_bass_kernel_spmd(nc, in_maps, core_ids=list(range(8)))
    LAST_RESULTS = res

    y = np.empty_like(x)
    for b in range(B):
        yT = (res.results[2 * b]["yT"].astype(np.float32)
              + res.results[2 * b + 1]["yT"].astype(np.float32))
        y[b] = yT.T
    return y
